# revision 36
# baseline (speedup 1.0000x reference)
"""Trainium2 Bass kernel for nn_Criterion (retrieval_knn, B=4 V=8192 F=16384 N=8192).

Per-core work (8 cores, data-parallel over B with 2-way split of N per batch):
  faces are processed in 8 pieces of 2048, pipelined: gpsimd ap_gather of
  piece t+1 overlaps DVE scoring of piece t.  Per piece: gather 3 corners
  (vertex table in the 8 Q7-core partitions), SBUF->SBUF redistribute to
  [P, 16, 3, 3], face math -> bf16 split rhs table caug_t[12, 2048] (rows
  [bh bm bh] of (-2c, |c|^2)) + gather-table rows tdram[piece].
  Scoring is block-major: for face-block t, all 32 point-chunks run a
  K=12 bf16-split matmul into PSUM and one fused custom-DVE pass that
  computes the running min AND argmin (codes -(OFS+idx) at running-min
  records, prefix-min elsewhere; accum=MIN chained across blocks via
  [P, CHUNKS] carry arrays).  No ScalarE Sign pass, no prefix array.
  finish: indirect-gather (n, c.n) per point, d = p.n - c.n,
  interp = relu(eps - d); per-partition partial sums out.
Host sums the 8x128x2 partials into (loss, perc).
"""

import numpy as np

B, V, F, N = 4, 8192, 16384, 8192
NCORES = 8
SPLIT = 2                 # cores per batch
NS = N // SPLIT           # 4096 points per core
P = 128
CHUNKS = NS // P          # 32
FT = 2048                 # faces per piece/block (4 PSUM banks)
NBLK = F // FT            # 8
FPQ = FT // P             # 16 faces per partition per piece
EPS = 1e-3
WEIGHT = 1000.0
BIG = 3.0e38
OFS = 1048576.0           # 2^20: argmin codes are -(OFS + face_idx)
VH = 5120                 # piece-0 faces only touch vertices < VH (host-sorted)
SIM_SAFE = False          # True: full vertex broadcast so CoreSim sees no
                          # uninitialized reads (HW only needs the 8 Q7 rows)

_CACHE = {}


def _register_ops():
    """Register (once) the two fused custom-DVE ops.

    ARGMIN_SCAN_ANT: m_k = min-scan(in0, carry=s0); out[k] = code when
      in0[k] <= m_k (a running-min record) else m_k, where
      code = -(imm2 + k); accum_out = min(s1, out[0..N-1]).  Since codes
      (-2^20-ish) lie far below every score, the chained accum yields the
      code of the LAST record = the argmin (last occurrence; deviates from
      first-occurrence only on exact fp32 ties).  out[N-1] is the block-end
      running min unless element N-1 itself recorded -- CARRY_FIX_ANT
      repairs that case from the raw score: carry = in1 if in0 < s0 else in0.
    """
    from concourse import dve_ops as D
    from concourse.dve_spec import (
        Spec, Src0, Src1, C0, C1, C2, Zero, Idx, scan, select, AluOp, lower,
    )

    def _col(v, Pn):
        a = np.asarray(v, np.float32)
        return a.reshape(Pn, 1) if a.size > 1 else np.broadcast_to(a.reshape(-1), (Pn, 1))

    def _argmin_ref(in0, in1, s0, s1, imm2):
        x = np.asarray(in0, np.float32)
        Pn = x.shape[0]
        x2 = x.reshape(Pn, -1)
        init = _col(s0, Pn)
        m = np.minimum.accumulate(np.concatenate([init, x2], 1), axis=1)[:, 1:]
        code = -(imm2 + np.arange(x2.shape[1], dtype=np.float32))[None, :]
        outv = np.where(x2 <= m, code, m).astype(np.float32)
        accum = np.minimum(_col(s1, Pn), outv.min(1, keepdims=True)).astype(np.float32)
        return outv.reshape(in0.shape), accum

    def _carryfix_ref(in0, in1, s0, s1, imm2):
        Pn = np.asarray(in0).shape[0]
        return np.where(
            np.asarray(in0, np.float32) < _col(s0, Pn).reshape(np.asarray(in0).shape),
            np.asarray(in1, np.float32), np.asarray(in0, np.float32),
        ).astype(np.float32)

    have = {op.name for op in D.OPS}
    defs = []
    if "ARGMIN_SCAN_ANT" not in have:
        m = scan(AluOp.MIN, Src0, init=C0)
        body = select(Src0 <= m, Zero - (C2 + Idx), m)
        defs.append(("ARGMIN_SCAN_ANT",
                     Spec(body=body, accum=AluOp.MIN, accum_init=C1,
                          reference=_argmin_ref)))
    if "CARRY_FIX_ANT" not in have:
        defs.append(("CARRY_FIX_ANT",
                     Spec(body=select(Src0 < C0, Src1, Src0),
                          reference=_carryfix_ref)))
    for name, spec in defs:
        op = D.DveOp(name, spec, subdim=False, uops_sha={})
        D.OPS.append(op)
        D._SUB_OPCODE_FOR_NAME[op.name] = D._CUSTOM_DVE_ROW_BASE + len(D.OPS) - 1
        D.CUSTOM_DVE_SPECS[op.name] = spec
        for ver in ("v3", "v4"):
            s = D.DveOpSpec(name=op.name, opcode=D.get_dve_sub_opcode(op.name),
                            uops=lower(spec, ver=ver), rd1_en=D.has_src1(spec))
            op.uops_sha[ver] = s.sha(ver)
    ops = {op.name: op for op in D.OPS}
    return ops["ARGMIN_SCAN_ANT"], ops["CARRY_FIX_ANT"]


def _build_nc():
    import concourse.mybir as mybir
    import concourse.tile as tile
    import concourse.bass as bass
    from concourse import bacc

    f32 = mybir.dt.float32
    i32 = mybir.dt.int32
    i16 = mybir.dt.int16
    bf = mybir.dt.bfloat16
    Alu = mybir.AluOpType
    Act = mybir.ActivationFunctionType
    X = mybir.AxisListType.X

    argmin_op, carryfix_op = _register_ops()

    nc = bacc.Bacc(None, target_bir_lowering=False)

    pred = nc.dram_tensor("pred", [NS, 3], f32, kind="ExternalInput")
    predT = nc.dram_tensor("predT", [3, NS], f32, kind="ExternalInput")
    opos = nc.dram_tensor("opos", [V, 3], f32, kind="ExternalInput")
    # per-piece gather index lists, host-packed (see _make_in_maps)
    gidx = nc.dram_tensor("gidx", [P, NBLK * 3 * FPQ], i16, kind="ExternalInput")
    out = nc.dram_tensor("out", [P, 2], f32, kind="ExternalOutput")
    tdram = nc.dram_tensor("tdram", [F, 4], f32, kind="Internal")
    caugd = [nc.dram_tensor(f"caug{t}", [12, FT], bf, kind="Internal")
             for t in range(NBLK)]

    NIDXQ = 3 * FPQ * 16      # 768 gathered rows per Q7 core per piece

    with tile.TileContext(nc) as tc:
        with tc.tile_pool(name="const1", bufs=1) as constp:
            # ---------- constants / lhs prep (overlaps the first gathers) ----------
            gidx_sb = constp.tile([P, NBLK * 3 * FPQ], i16)
            nc.sync.dma_start(out=gidx_sb[:], in_=gidx[:])
            vtx = constp.tile([P, V, 3], f32)
            if SIM_SAFE:
                vtx_bcast = bass.AP(opos[:].tensor, 0, [[0, P], [3, V], [1, 3]])
                nc.sync.dma_start(out=vtx[:], in_=vtx_bcast)
            else:
                # split strided broadcast: low-vertex prefix first so the
                # piece-0 gather (host sorts piece-0 faces to vertices < VH)
                # starts before the full table lands
                srcA = bass.AP(opos[:].tensor, 0, [[0, 8], [3, VH], [1, 3]])
                nc.sync.dma_start(out=vtx[0:128:16, 0:VH, :], in_=srcA)
                srcB = bass.AP(opos[:].tensor, VH * 3, [[0, 8], [3, V - VH], [1, 3]])
                nc.scalar.dma_start(out=vtx[0:128:16, VH:V, :], in_=srcB)
            # lhs rows [ah, ah, am] paired with rhs rows [bh, bm, bh]
            paug = constp.tile([12, NS], bf)
            with tc.tile_pool(name="ptmp", bufs=1) as ptmp:
                paug0 = ptmp.tile([4, NS], f32)
                nc.vector.memset(paug0[:], 1.0)
                nc.sync.dma_start(out=paug0[0:3, :], in_=predT[:])
                ph16 = ptmp.tile([4, NS], bf)
                nc.vector.tensor_copy(out=ph16[:], in_=paug0[:])
                phf = ptmp.tile([4, NS], f32)
                nc.vector.tensor_copy(out=phf[:], in_=ph16[:])
                pr1 = ptmp.tile([4, NS], f32)
                nc.vector.tensor_tensor(out=pr1[:], in0=paug0[:], in1=phf[:], op=Alu.subtract)
                pm16 = ptmp.tile([4, NS], bf)
                nc.vector.tensor_copy(out=pm16[:], in_=pr1[:])
                for i, part in enumerate((ph16, ph16, pm16)):
                    nc.sync.dma_start(out=paug[4 * i:4 * i + 4, :], in_=part[:])
            pred_pts = constp.tile([P, CHUNKS, 3], f32)
            nc.sync.dma_start(
                out=pred_pts[:], in_=pred[:].rearrange("(k p) j -> p k j", p=P)
            )
            negofs = constp.tile([P, 1], f32)
            nc.vector.memset(negofs[:], -OFS)
            tiny = constp.tile([P, 1], f32)
            nc.vector.memset(tiny[:], 1e-24)
            idx_all = constp.tile([P, CHUNKS], i32)
            g = constp.tile([P, CHUNKS, 4], f32)
            # block-chain carry arrays
            runmin = constp.tile([P, CHUNKS], f32)
            chains = [constp.tile([P, CHUNKS], f32, name=f"chain{i}") for i in range(2)]
            caug_sb = [constp.tile([12, FT], bf, name=f"caug_sb{t}") for t in range(NBLK)]

            def gather_piece(gp, t, dest, off):
                """ap_gather piece t and redistribute into face-slot window
                [off*FPQ, (off+1)*FPQ) of batch tile `dest`."""
                vg = gp.tile([P, NIDXQ, 3], f32, tag="vg")
                ne = VH if t == 0 else V
                nc.gpsimd.ap_gather(
                    vg[:], vtx[:, 0:ne, :], gidx_sb[:, t * 3 * FPQ:(t + 1) * 3 * FPQ],
                    channels=P, num_elems=ne, d=3, num_idxs=NIDXQ,
                )
                for c in range(8):
                    nc.sync.dma_start(
                        out=dest[16 * c:16 * c + 16, off * FPQ:(off + 1) * FPQ, :, :],
                        in_=vg[16 * c:16 * c + 1, :, :],
                    )

            def piece_tables(mp, vtx9, t0, npq):
                """face math for pieces [t0, t0+npq) (batched so the fixed
                per-op DVE cost amortizes) -> caug_sb[t] + tdram rows."""
                ve = nc.vector
                FQ = FPQ * npq
                v0 = vtx9[:, :, 0, :]
                v1 = vtx9[:, :, 1, :]
                v2 = vtx9[:, :, 2, :]
                cc = mp.tile([P, FQ, 3], f32, tag=f"cc{npq}")
                ve.tensor_tensor(out=cc[:], in0=v0, in1=v1, op=Alu.add)
                ve.tensor_tensor(out=cc[:], in0=cc[:], in1=v2, op=Alu.add)
                ve.tensor_scalar_mul(cc[:], cc[:], 1.0 / 3.0)
                e1 = mp.tile([P, FQ, 3], f32, tag=f"e1{npq}")
                e2 = mp.tile([P, FQ, 3], f32, tag=f"e2{npq}")
                ve.tensor_tensor(out=e1[:], in0=v1, in1=v0, op=Alu.subtract)
                ve.tensor_tensor(out=e2[:], in0=v2, in1=v0, op=Alu.subtract)
                tint = mp.tile([P, FQ, 4], f32, tag=f"tint{npq}")
                tmp = mp.tile([P, FQ], f32, tag=f"tmp{npq}")
                tmp2 = mp.tile([P, FQ], f32, tag=f"tmp2{npq}")
                for j in range(3):
                    a, b2 = (j + 1) % 3, (j + 2) % 3
                    ve.tensor_tensor(out=tmp[:], in0=e1[:, :, a], in1=e2[:, :, b2], op=Alu.mult)
                    ve.tensor_tensor(out=tmp2[:], in0=e1[:, :, b2], in1=e2[:, :, a], op=Alu.mult)
                    ve.tensor_tensor(out=tint[:, :, j], in0=tmp[:], in1=tmp2[:], op=Alu.subtract)
                nn2 = mp.tile([P, FQ], f32, tag=f"nn2{npq}")
                ve.tensor_tensor(out=nn2[:], in0=tint[:, :, 0], in1=tint[:, :, 0], op=Alu.mult)
                for j in (1, 2):
                    ve.tensor_tensor(out=tmp[:], in0=tint[:, :, j], in1=tint[:, :, j], op=Alu.mult)
                    ve.tensor_tensor(out=nn2[:], in0=nn2[:], in1=tmp[:], op=Alu.add)
                # rinv = 1/sqrt(nn2 + tiny)  (reference clamps |n| at 1e-12)
                nc.scalar.activation(out=tmp[:], in_=nn2[:], func=Act.Sqrt,
                                     bias=tiny[:, 0:1], scale=1.0)
                nc.vector.reciprocal(tmp2[:], tmp[:])
                for j in range(3):
                    ve.tensor_tensor(out=tint[:, :, j], in0=tint[:, :, j], in1=tmp2[:], op=Alu.mult)
                ve.tensor_tensor(out=tmp[:], in0=cc[:, :, 0], in1=tint[:, :, 0], op=Alu.mult)
                ve.tensor_tensor(out=tmp2[:], in0=cc[:, :, 1], in1=tint[:, :, 1], op=Alu.mult)
                ve.tensor_tensor(out=tmp[:], in0=tmp[:], in1=tmp2[:], op=Alu.add)
                ve.tensor_tensor(out=tmp2[:], in0=cc[:, :, 2], in1=tint[:, :, 2], op=Alu.mult)
                ve.tensor_tensor(out=tint[:, :, 3], in0=tmp[:], in1=tmp2[:], op=Alu.add)
                cpl = mp.tile([P, 4, FQ], f32, tag=f"cpl{npq}")
                for j in range(3):
                    ve.tensor_copy(out=cpl[:, j, :], in_=cc[:, :, j])
                ve.tensor_scalar_mul(cpl[:, 0:3, :], cpl[:, 0:3, :], -2.0)
                ve.tensor_tensor(out=cpl[:, 3, :], in0=cc[:, :, 0], in1=cc[:, :, 0], op=Alu.mult)
                for j in (1, 2):
                    ve.tensor_tensor(out=tmp[:], in0=cc[:, :, j], in1=cc[:, :, j], op=Alu.mult)
                    ve.tensor_tensor(out=cpl[:, 3, :], in0=cpl[:, 3, :], in1=tmp[:], op=Alu.add)
                for j in range(npq):
                    t = t0 + j
                    nc.sync.dma_start(
                        out=tdram[t * FT:(t + 1) * FT, :].rearrange("(p i) k -> p i k", p=P),
                        in_=tint[:, j * FPQ:(j + 1) * FPQ, :],
                    )
                # bf16 split rows [bh, bm, bh]
                ch16 = mp.tile([P, 4, FQ], bf, tag=f"ch16{npq}")
                ve.tensor_copy(out=ch16[:], in_=cpl[:])
                chf = mp.tile([P, 4, FQ], f32, tag=f"chf{npq}")
                ve.tensor_copy(out=chf[:], in_=ch16[:])
                cr1 = mp.tile([P, 4, FQ], f32, tag=f"cr1{npq}")
                ve.tensor_tensor(out=cr1[:], in0=cpl[:], in1=chf[:], op=Alu.subtract)
                cm16 = mp.tile([P, 4, FQ], bf, tag=f"cm16{npq}")
                ve.tensor_copy(out=cm16[:], in_=cr1[:])
                cpl12 = mp.tile([P, 12, FQ], bf, tag=f"cpl12{npq}")
                for i, part in enumerate((ch16, cm16, ch16)):
                    ve.tensor_copy(out=cpl12[:, 4 * i:4 * i + 4, :], in_=part[:])
                for j in range(npq):
                    t = t0 + j
                    nc.scalar.dma_start(
                        out=caugd[t][:].rearrange("r (p i) -> p r i", p=P),
                        in_=cpl12[:, :, j * FPQ:(j + 1) * FPQ],
                    )
                    nc.scalar.dma_start(out=caug_sb[t][:], in_=caugd[t][:])

            with (
                tc.tile_pool(name="gp", bufs=2) as gp,
                tc.tile_pool(name="g9", bufs=1) as g9,
                tc.tile_pool(name="mp", bufs=1) as mp,
                tc.tile_pool(name="psump", bufs=2, space="PSUM") as psump,
                tc.tile_pool(name="dumpp", bufs=2) as dumpp,
                tc.tile_pool(name="smallp", bufs=4) as smallp,
            ):
                # math batches: piece 0 alone (fast start), then {1,2}, {3..7}
                # (bigger DVE ops amortize the fixed per-op cost); batch math is
                # emitted mid-stream at points where its gathers have landed.
                batches = [(0, 1), (1, 2), (3, 5)]
                v9 = {t0: g9.tile([P, FPQ * npq, 3, 3], f32, name=f"v9_{t0}")
                      for t0, npq in batches}
                # emit all gathers up-front; they self-pipeline on the gpsimd
                # queue via the vg ring
                for t0, npq in batches:
                    for j in range(npq):
                        gather_piece(gp, t0 + j, v9[t0], j)
                piece_tables(mp, v9[0], 0, 1)
                for t in range(NBLK):
                    for k in range(CHUNKS):
                        if t == 0 and k == 12:
                            piece_tables(mp, v9[1], 1, 2)
                        if t == 2 and k == 4:
                            piece_tables(mp, v9[3], 3, 5)
                        lhsT = paug[:, k * P:(k + 1) * P]
                        ps = psump.tile([P, FT], f32, tag="ps")
                        for j in range(FT // 512):
                            rr = caug_sb[t][:, j * 512:(j + 1) * 512]
                            nc.tensor.matmul(
                                ps[:, j * 512:(j + 1) * 512],
                                lhsT,
                                rr,
                                start=True,
                                stop=True,
                            )
                        dump = dumpp.tile([P, FT], f32, tag="dump")
                        if t < NBLK - 1:
                            # stash the raw last score in SBUF (ScalarE, off the
                            # PSUM-reuse loop) so CARRY_FIX never touches PSUM
                            pscp = smallp.tile([P, 1], f32, tag="pscp")
                            nc.scalar.copy(out=pscp[:], in_=ps[:, FT - 1:FT])
                        nc.vector._custom_dve(
                            argmin_op,
                            out=dump[:],
                            in0=ps[:],
                            s0=BIG if t == 0 else runmin[:, k:k + 1],
                            s1=BIG if t == 0 else chains[(t - 1) % 2][:, k:k + 1],
                            imm2=OFS + t * FT,
                            accum_out=chains[t % 2][:, k:k + 1],
                        )
                        if t < NBLK - 1:
                            nc.vector._custom_dve(
                                carryfix_op,
                                out=runmin[:, k:k + 1],
                                in0=dump[:, FT - 1:FT],
                                in1=pscp[:],
                                s0=-OFS / 2,
                            )
                        else:
                            # decode idx = -chain - OFS on ScalarE (keeps the
                            # Vector queue pure argmin)
                            nc.scalar.activation(
                                out=idx_all[:, k:k + 1],
                                in_=chains[t % 2][:, k:k + 1],
                                func=Act.Copy, bias=-OFS, scale=-1.0,
                            )
                            nc.gpsimd.indirect_dma_start(
                                out=g[:, k, :],
                                out_offset=None,
                                in_=tdram[:],
                                in_offset=bass.IndirectOffsetOnAxis(
                                    ap=idx_all[:, k:k + 1], axis=0),
                            )

            # ---------------- finish ----------------
            with tc.tile_pool(name="finp", bufs=1) as finp:
                prod = finp.tile([P, CHUNKS, 3], f32)
                nc.vector.tensor_tensor(out=prod[:], in0=g[:, :, 0:3], in1=pred_pts[:], op=Alu.mult)
                s3 = finp.tile([P, CHUNKS], f32)
                nc.vector.tensor_reduce(out=s3[:], in_=prod[:], axis=X, op=Alu.add)
                d = finp.tile([P, CHUNKS], f32)
                nc.vector.tensor_tensor(out=d[:], in0=s3[:], in1=g[:, :, 3], op=Alu.subtract)
                interp = finp.tile([P, CHUNKS], f32)
                eps1 = finp.tile([P, 1], f32)
                nc.vector.memset(eps1[:], EPS)
                nc.scalar.activation(out=interp[:], in_=d[:], func=Act.Relu, bias=eps1[:, 0:1], scale=-1.0)
                outsb = finp.tile([P, 2], f32)
                sgn = finp.tile([P, CHUNKS], f32)
                nc.scalar.activation(
                    out=sgn[:], in_=interp[:], func=Act.Sign, bias=0.0, scale=1.0,
                    accum_out=outsb[:, 1:2],
                )
                sq = finp.tile([P, CHUNKS], f32)
                nc.scalar.square(sq[:], interp[:])
                cube = finp.tile([P, CHUNKS], f32)
                nc.vector.tensor_tensor(out=cube[:], in0=sq[:], in1=interp[:], op=Alu.mult)
                nc.vector.tensor_reduce(out=outsb[:, 0:1], in_=cube[:], axis=X, op=Alu.add)
                nc.sync.dma_start(out=out[:], in_=outsb[:])

    nc.compile()
    return nc


def _get_nc():
    if "nc" not in _CACHE:
        _CACHE["nc"] = _build_nc()
    return _CACHE["nc"]


def _make_in_maps(pred_pos, obstacle_pos, obstacle_faces):
    pred_pos = np.ascontiguousarray(np.asarray(pred_pos, dtype=np.float32))
    obstacle_pos = np.ascontiguousarray(np.asarray(obstacle_pos, dtype=np.float32))
    faces = np.asarray(obstacle_faces).astype(np.int64)
    in_maps = []
    for c in range(NCORES):
        b, half = c // SPLIT, c % SPLIT
        pr = np.ascontiguousarray(pred_pos[b, half * NS:(half + 1) * NS])
        # gather-order indices, per piece t of 2048 faces: Q7 core q handles
        # faces [t*2048 + q*256, +256); list order j = (part, slot, corner);
        # round-robin wrapped in the core's 16 partitions:
        # gidx[16q+r, t*48 + m] = list[m*16 + r].
        fb = faces[b][np.argsort(faces[b].max(1), kind="stable")]
        assert int(fb[:2048].max()) < VH, "piece-0 vertex bound violated"
        fc = fb.reshape(NBLK, 8, 3 * FPQ * 16)   # piece, core, (q,i,k)
        gx = np.zeros((P, NBLK * 3 * FPQ), np.int16)
        for t in range(NBLK):
            for q in range(8):
                gx[16 * q:16 * q + 16, t * 3 * FPQ:(t + 1) * 3 * FPQ] = (
                    fc[t, q].reshape(3 * FPQ, 16).T.astype(np.int16)
                )
        in_maps.append({
            "pred": pr,
            "predT": np.ascontiguousarray(pr.T),
            "opos": obstacle_pos[b],
            "gidx": gx,
        })
    return in_maps


def kernel(pred_pos, obstacle_pos, obstacle_faces):
    from concourse.bass_utils import run_bass_kernel_spmd

    nc = _get_nc()
    in_maps = _make_in_maps(pred_pos, obstacle_pos, obstacle_faces)
    res = run_bass_kernel_spmd(nc, in_maps, core_ids=list(range(NCORES)))
    outs = np.stack([r["out"] for r in res.results])  # [8, 128, 2]
    loss_sum = float(outs[:, :, 0].astype(np.float64).sum())
    cnt_sum = float(outs[:, :, 1].astype(np.float64).sum())
    loss = np.float32(loss_sum / B * WEIGHT)
    perc = np.float32(cnt_sum / (B * N))
    return loss, perc


# revision 37
# speedup vs baseline: 1.0094x; 1.0094x over previous
"""Trainium2 Bass kernel for nn_Criterion (retrieval_knn, B=4 V=8192 F=16384 N=8192).

Per-core work (8 cores, data-parallel over B with 2-way split of N per batch):
  faces are processed in 8 pieces of 2048, pipelined: gpsimd ap_gather of
  piece t+1 overlaps DVE scoring of piece t.  Per piece: gather 3 corners
  (vertex table in the 8 Q7-core partitions), SBUF->SBUF redistribute to
  [P, 16, 3, 3], face math -> bf16 split rhs table caug_t[12, 2048] (rows
  [bh bm bh] of (-2c, |c|^2)) + gather-table rows tdram[piece].
  Scoring is block-major: for face-block t, all 32 point-chunks run a
  K=12 bf16-split matmul into PSUM and one fused custom-DVE pass that
  computes the running min AND argmin (codes -(OFS+idx) at running-min
  records, prefix-min elsewhere; accum=MIN chained across blocks via
  [P, CHUNKS] carry arrays).  No ScalarE Sign pass, no prefix array.
  finish: indirect-gather (n, c.n) per point, d = p.n - c.n,
  interp = relu(eps - d); per-partition partial sums out.
Host sums the 8x128x2 partials into (loss, perc).
"""

import numpy as np

B, V, F, N = 4, 8192, 16384, 8192
NCORES = 8
SPLIT = 2                 # cores per batch
NS = N // SPLIT           # 4096 points per core
P = 128
CHUNKS = NS // P          # 32
FT = 2048                 # faces per piece/block (4 PSUM banks)
NBLK = F // FT            # 8
FPQ = FT // P             # 16 faces per partition per piece
EPS = 1e-3
WEIGHT = 1000.0
BIG = 3.0e38
OFS = 1048576.0           # 2^20: argmin codes are -(OFS + face_idx)
SIM_SAFE = False          # True: full vertex broadcast so CoreSim sees no
                          # uninitialized reads (HW only needs the 8 Q7 rows)

_CACHE = {}


def _register_ops():
    """Register (once) the two fused custom-DVE ops.

    ARGMIN_SCAN_ANT: m_k = min-scan(in0, carry=s0); out[k] = code when
      in0[k] <= m_k (a running-min record) else m_k, where
      code = -(imm2 + k); accum_out = min(s1, out[0..N-1]).  Since codes
      (-2^20-ish) lie far below every score, the chained accum yields the
      code of the LAST record = the argmin (last occurrence; deviates from
      first-occurrence only on exact fp32 ties).  out[N-1] is the block-end
      running min unless element N-1 itself recorded -- CARRY_FIX_ANT
      repairs that case from the raw score: carry = in1 if in0 < s0 else in0.
    """
    from concourse import dve_ops as D
    from concourse.dve_spec import (
        Spec, Src0, Src1, C0, C1, C2, Zero, Idx, scan, select, AluOp, lower,
    )

    def _col(v, Pn):
        a = np.asarray(v, np.float32)
        return a.reshape(Pn, 1) if a.size > 1 else np.broadcast_to(a.reshape(-1), (Pn, 1))

    def _argmin_ref(in0, in1, s0, s1, imm2):
        x = np.asarray(in0, np.float32)
        Pn = x.shape[0]
        x2 = x.reshape(Pn, -1)
        init = _col(s0, Pn)
        m = np.minimum.accumulate(np.concatenate([init, x2], 1), axis=1)[:, 1:]
        code = -(imm2 + np.arange(x2.shape[1], dtype=np.float32))[None, :]
        outv = np.where(x2 <= m, code, m).astype(np.float32)
        accum = np.minimum(_col(s1, Pn), outv.min(1, keepdims=True)).astype(np.float32)
        return outv.reshape(in0.shape), accum

    def _carryfix_ref(in0, in1, s0, s1, imm2):
        Pn = np.asarray(in0).shape[0]
        return np.where(
            np.asarray(in0, np.float32) < _col(s0, Pn).reshape(np.asarray(in0).shape),
            np.asarray(in1, np.float32), np.asarray(in0, np.float32),
        ).astype(np.float32)

    have = {op.name for op in D.OPS}
    defs = []
    if "ARGMIN_SCAN_ANT" not in have:
        m = scan(AluOp.MIN, Src0, init=C0)
        body = select(Src0 <= m, Zero - (C2 + Idx), m)
        defs.append(("ARGMIN_SCAN_ANT",
                     Spec(body=body, accum=AluOp.MIN, accum_init=C1,
                          reference=_argmin_ref)))
    if "CARRY_FIX_ANT" not in have:
        defs.append(("CARRY_FIX_ANT",
                     Spec(body=select(Src0 < C0, Src1, Src0),
                          reference=_carryfix_ref)))
    for name, spec in defs:
        op = D.DveOp(name, spec, subdim=False, uops_sha={})
        D.OPS.append(op)
        D._SUB_OPCODE_FOR_NAME[op.name] = D._CUSTOM_DVE_ROW_BASE + len(D.OPS) - 1
        D.CUSTOM_DVE_SPECS[op.name] = spec
        for ver in ("v3", "v4"):
            s = D.DveOpSpec(name=op.name, opcode=D.get_dve_sub_opcode(op.name),
                            uops=lower(spec, ver=ver), rd1_en=D.has_src1(spec))
            op.uops_sha[ver] = s.sha(ver)
    ops = {op.name: op for op in D.OPS}
    return ops["ARGMIN_SCAN_ANT"], ops["CARRY_FIX_ANT"]


def _build_nc():
    import concourse.mybir as mybir
    import concourse.tile as tile
    import concourse.bass as bass
    from concourse import bacc

    f32 = mybir.dt.float32
    i32 = mybir.dt.int32
    i16 = mybir.dt.int16
    bf = mybir.dt.bfloat16
    Alu = mybir.AluOpType
    Act = mybir.ActivationFunctionType
    X = mybir.AxisListType.X

    argmin_op, carryfix_op = _register_ops()

    nc = bacc.Bacc(None, target_bir_lowering=False)

    pred = nc.dram_tensor("pred", [NS, 3], f32, kind="ExternalInput")
    predT = nc.dram_tensor("predT", [3, NS], f32, kind="ExternalInput")
    opos = nc.dram_tensor("opos", [V, 3], f32, kind="ExternalInput")
    # per-piece gather index lists, host-packed (see _make_in_maps)
    gidx = nc.dram_tensor("gidx", [P, NBLK * 3 * FPQ], i16, kind="ExternalInput")
    out = nc.dram_tensor("out", [P, 2], f32, kind="ExternalOutput")
    tdram = nc.dram_tensor("tdram", [F, 4], f32, kind="Internal")
    caugd = [nc.dram_tensor(f"caug{t}", [12, FT], bf, kind="Internal")
             for t in range(NBLK)]

    NIDXQ = 3 * FPQ * 16      # 768 gathered rows per Q7 core per piece

    with tile.TileContext(nc) as tc:
        with tc.tile_pool(name="const1", bufs=1) as constp:
            # ---------- constants / lhs prep (overlaps the first gathers) ----------
            gidx_sb = constp.tile([P, NBLK * 3 * FPQ], i16)
            nc.sync.dma_start(out=gidx_sb[:], in_=gidx[:])
            vtx = constp.tile([P, V, 3], f32)
            if SIM_SAFE:
                vtx_bcast = bass.AP(opos[:].tensor, 0, [[0, P], [3, V], [1, 3]])
                nc.sync.dma_start(out=vtx[:], in_=vtx_bcast)
            else:
                # one strided-partition broadcast: the DMA lowering fans the
                # 8x96KB replication out across many hardware queues
                src8 = bass.AP(opos[:].tensor, 0, [[0, 8], [3, V], [1, 3]])
                nc.sync.dma_start(out=vtx[0:128:16, :, :], in_=src8)
            # lhs rows [ah, ah, am] paired with rhs rows [bh, bm, bh]
            paug = constp.tile([12, NS], bf)
            with tc.tile_pool(name="ptmp", bufs=1) as ptmp:
                paug0 = ptmp.tile([4, NS], f32)
                nc.vector.memset(paug0[:], 1.0)
                nc.sync.dma_start(out=paug0[0:3, :], in_=predT[:])
                ph16 = ptmp.tile([4, NS], bf)
                nc.vector.tensor_copy(out=ph16[:], in_=paug0[:])
                phf = ptmp.tile([4, NS], f32)
                nc.vector.tensor_copy(out=phf[:], in_=ph16[:])
                pr1 = ptmp.tile([4, NS], f32)
                nc.vector.tensor_tensor(out=pr1[:], in0=paug0[:], in1=phf[:], op=Alu.subtract)
                pm16 = ptmp.tile([4, NS], bf)
                nc.vector.tensor_copy(out=pm16[:], in_=pr1[:])
                for i, part in enumerate((ph16, ph16, pm16)):
                    nc.sync.dma_start(out=paug[4 * i:4 * i + 4, :], in_=part[:])
            pred_pts = constp.tile([P, CHUNKS, 3], f32)
            nc.sync.dma_start(
                out=pred_pts[:], in_=pred[:].rearrange("(k p) j -> p k j", p=P)
            )
            negofs = constp.tile([P, 1], f32)
            nc.vector.memset(negofs[:], -OFS)
            tiny = constp.tile([P, 1], f32)
            nc.vector.memset(tiny[:], 1e-24)
            idx_all = constp.tile([P, CHUNKS], i32)
            g = constp.tile([P, CHUNKS, 4], f32)
            # block-chain carry arrays
            runmin = constp.tile([P, CHUNKS], f32)
            chains = [constp.tile([P, CHUNKS], f32, name=f"chain{i}") for i in range(2)]
            caug_sb = [constp.tile([12, FT], bf, name=f"caug_sb{t}") for t in range(NBLK)]

            def gather_piece(gp, t, dest, off):
                """ap_gather piece t and redistribute into face-slot window
                [off*FPQ, (off+1)*FPQ) of batch tile `dest`."""
                vg = gp.tile([P, NIDXQ, 3], f32, tag="vg")
                nc.gpsimd.ap_gather(
                    vg[:], vtx[:], gidx_sb[:, t * 3 * FPQ:(t + 1) * 3 * FPQ],
                    channels=P, num_elems=V, d=3, num_idxs=NIDXQ,
                )
                for c in range(8):
                    nc.sync.dma_start(
                        out=dest[16 * c:16 * c + 16, off * FPQ:(off + 1) * FPQ, :, :],
                        in_=vg[16 * c:16 * c + 1, :, :],
                    )

            def piece_tables(mp, vtx9, t0, npq):
                """face math for pieces [t0, t0+npq) (batched so the fixed
                per-op DVE cost amortizes) -> caug_sb[t] + tdram rows."""
                ve = nc.vector
                FQ = FPQ * npq
                v0 = vtx9[:, :, 0, :]
                v1 = vtx9[:, :, 1, :]
                v2 = vtx9[:, :, 2, :]
                cc = mp.tile([P, FQ, 3], f32, tag=f"cc{npq}")
                ve.tensor_tensor(out=cc[:], in0=v0, in1=v1, op=Alu.add)
                ve.tensor_tensor(out=cc[:], in0=cc[:], in1=v2, op=Alu.add)
                ve.tensor_scalar_mul(cc[:], cc[:], 1.0 / 3.0)
                e1 = mp.tile([P, FQ, 3], f32, tag=f"e1{npq}")
                e2 = mp.tile([P, FQ, 3], f32, tag=f"e2{npq}")
                ve.tensor_tensor(out=e1[:], in0=v1, in1=v0, op=Alu.subtract)
                ve.tensor_tensor(out=e2[:], in0=v2, in1=v0, op=Alu.subtract)
                tint = mp.tile([P, FQ, 4], f32, tag=f"tint{npq}")
                tmp = mp.tile([P, FQ], f32, tag=f"tmp{npq}")
                tmp2 = mp.tile([P, FQ], f32, tag=f"tmp2{npq}")
                for j in range(3):
                    a, b2 = (j + 1) % 3, (j + 2) % 3
                    ve.tensor_tensor(out=tmp[:], in0=e1[:, :, a], in1=e2[:, :, b2], op=Alu.mult)
                    ve.tensor_tensor(out=tmp2[:], in0=e1[:, :, b2], in1=e2[:, :, a], op=Alu.mult)
                    ve.tensor_tensor(out=tint[:, :, j], in0=tmp[:], in1=tmp2[:], op=Alu.subtract)
                nn2 = mp.tile([P, FQ], f32, tag=f"nn2{npq}")
                ve.tensor_tensor(out=nn2[:], in0=tint[:, :, 0], in1=tint[:, :, 0], op=Alu.mult)
                for j in (1, 2):
                    ve.tensor_tensor(out=tmp[:], in0=tint[:, :, j], in1=tint[:, :, j], op=Alu.mult)
                    ve.tensor_tensor(out=nn2[:], in0=nn2[:], in1=tmp[:], op=Alu.add)
                # rinv = 1/sqrt(nn2 + tiny)  (reference clamps |n| at 1e-12)
                nc.scalar.activation(out=tmp[:], in_=nn2[:], func=Act.Sqrt,
                                     bias=tiny[:, 0:1], scale=1.0)
                nc.vector.reciprocal(tmp2[:], tmp[:])
                for j in range(3):
                    ve.tensor_tensor(out=tint[:, :, j], in0=tint[:, :, j], in1=tmp2[:], op=Alu.mult)
                ve.tensor_tensor(out=tmp[:], in0=cc[:, :, 0], in1=tint[:, :, 0], op=Alu.mult)
                ve.tensor_tensor(out=tmp2[:], in0=cc[:, :, 1], in1=tint[:, :, 1], op=Alu.mult)
                ve.tensor_tensor(out=tmp[:], in0=tmp[:], in1=tmp2[:], op=Alu.add)
                ve.tensor_tensor(out=tmp2[:], in0=cc[:, :, 2], in1=tint[:, :, 2], op=Alu.mult)
                ve.tensor_tensor(out=tint[:, :, 3], in0=tmp[:], in1=tmp2[:], op=Alu.add)
                cpl = mp.tile([P, 4, FQ], f32, tag=f"cpl{npq}")
                for j in range(3):
                    ve.tensor_copy(out=cpl[:, j, :], in_=cc[:, :, j])
                ve.tensor_scalar_mul(cpl[:, 0:3, :], cpl[:, 0:3, :], -2.0)
                ve.tensor_tensor(out=cpl[:, 3, :], in0=cc[:, :, 0], in1=cc[:, :, 0], op=Alu.mult)
                for j in (1, 2):
                    ve.tensor_tensor(out=tmp[:], in0=cc[:, :, j], in1=cc[:, :, j], op=Alu.mult)
                    ve.tensor_tensor(out=cpl[:, 3, :], in0=cpl[:, 3, :], in1=tmp[:], op=Alu.add)
                for j in range(npq):
                    t = t0 + j
                    nc.sync.dma_start(
                        out=tdram[t * FT:(t + 1) * FT, :].rearrange("(p i) k -> p i k", p=P),
                        in_=tint[:, j * FPQ:(j + 1) * FPQ, :],
                    )
                # bf16 split rows [bh, bm, bh]
                ch16 = mp.tile([P, 4, FQ], bf, tag=f"ch16{npq}")
                ve.tensor_copy(out=ch16[:], in_=cpl[:])
                chf = mp.tile([P, 4, FQ], f32, tag=f"chf{npq}")
                ve.tensor_copy(out=chf[:], in_=ch16[:])
                cr1 = mp.tile([P, 4, FQ], f32, tag=f"cr1{npq}")
                ve.tensor_tensor(out=cr1[:], in0=cpl[:], in1=chf[:], op=Alu.subtract)
                cm16 = mp.tile([P, 4, FQ], bf, tag=f"cm16{npq}")
                ve.tensor_copy(out=cm16[:], in_=cr1[:])
                cpl12 = mp.tile([P, 12, FQ], bf, tag=f"cpl12{npq}")
                for i, part in enumerate((ch16, cm16, ch16)):
                    ve.tensor_copy(out=cpl12[:, 4 * i:4 * i + 4, :], in_=part[:])
                for j in range(npq):
                    t = t0 + j
                    nc.scalar.dma_start(
                        out=caugd[t][:].rearrange("r (p i) -> p r i", p=P),
                        in_=cpl12[:, :, j * FPQ:(j + 1) * FPQ],
                    )
                    nc.scalar.dma_start(out=caug_sb[t][:], in_=caugd[t][:])

            with (
                tc.tile_pool(name="gp", bufs=2) as gp,
                tc.tile_pool(name="g9", bufs=1) as g9,
                tc.tile_pool(name="mp", bufs=1) as mp,
                tc.tile_pool(name="psump", bufs=2, space="PSUM") as psump,
                tc.tile_pool(name="dumpp", bufs=2) as dumpp,
                tc.tile_pool(name="smallp", bufs=4) as smallp,
            ):
                # math batches: piece 0 alone (fast start), then {1,2}, {3..7}
                # (bigger DVE ops amortize the fixed per-op cost); batch math is
                # emitted mid-stream at points where its gathers have landed.
                batches = [(0, 1), (1, 2), (3, 5)]
                v9 = {t0: g9.tile([P, FPQ * npq, 3, 3], f32, name=f"v9_{t0}")
                      for t0, npq in batches}
                # emit all gathers up-front; they self-pipeline on the gpsimd
                # queue via the vg ring
                for t0, npq in batches:
                    for j in range(npq):
                        gather_piece(gp, t0 + j, v9[t0], j)
                piece_tables(mp, v9[0], 0, 1)
                for t in range(NBLK):
                    for k in range(CHUNKS):
                        if t == 0 and k == 12:
                            piece_tables(mp, v9[1], 1, 2)
                        if t == 2 and k == 4:
                            piece_tables(mp, v9[3], 3, 5)
                        lhsT = paug[:, k * P:(k + 1) * P]
                        ps = psump.tile([P, FT], f32, tag="ps")
                        for j in range(FT // 512):
                            rr = caug_sb[t][:, j * 512:(j + 1) * 512]
                            nc.tensor.matmul(
                                ps[:, j * 512:(j + 1) * 512],
                                lhsT,
                                rr,
                                start=True,
                                stop=True,
                            )
                        dump = dumpp.tile([P, FT], f32, tag="dump")
                        if t < NBLK - 1:
                            # stash the raw last score in SBUF (ScalarE, off the
                            # PSUM-reuse loop) so CARRY_FIX never touches PSUM
                            pscp = smallp.tile([P, 1], f32, tag="pscp")
                            nc.scalar.copy(out=pscp[:], in_=ps[:, FT - 1:FT])
                        nc.vector._custom_dve(
                            argmin_op,
                            out=dump[:],
                            in0=ps[:],
                            s0=BIG if t == 0 else runmin[:, k:k + 1],
                            s1=BIG if t == 0 else chains[(t - 1) % 2][:, k:k + 1],
                            imm2=OFS + t * FT,
                            accum_out=chains[t % 2][:, k:k + 1],
                        )
                        if t < NBLK - 1:
                            nc.vector._custom_dve(
                                carryfix_op,
                                out=runmin[:, k:k + 1],
                                in0=dump[:, FT - 1:FT],
                                in1=pscp[:],
                                s0=-OFS / 2,
                            )
                        else:
                            # decode idx = -chain - OFS on ScalarE (keeps the
                            # Vector queue pure argmin)
                            nc.scalar.activation(
                                out=idx_all[:, k:k + 1],
                                in_=chains[t % 2][:, k:k + 1],
                                func=Act.Copy, bias=-OFS, scale=-1.0,
                            )
                            nc.gpsimd.indirect_dma_start(
                                out=g[:, k, :],
                                out_offset=None,
                                in_=tdram[:],
                                in_offset=bass.IndirectOffsetOnAxis(
                                    ap=idx_all[:, k:k + 1], axis=0),
                            )

            # ---------------- finish ----------------
            with tc.tile_pool(name="finp", bufs=1) as finp:
                prod = finp.tile([P, CHUNKS, 3], f32)
                nc.vector.tensor_tensor(out=prod[:], in0=g[:, :, 0:3], in1=pred_pts[:], op=Alu.mult)
                s3 = finp.tile([P, CHUNKS], f32)
                nc.vector.tensor_reduce(out=s3[:], in_=prod[:], axis=X, op=Alu.add)
                d = finp.tile([P, CHUNKS], f32)
                nc.vector.tensor_tensor(out=d[:], in0=s3[:], in1=g[:, :, 3], op=Alu.subtract)
                interp = finp.tile([P, CHUNKS], f32)
                eps1 = finp.tile([P, 1], f32)
                nc.vector.memset(eps1[:], EPS)
                nc.scalar.activation(out=interp[:], in_=d[:], func=Act.Relu, bias=eps1[:, 0:1], scale=-1.0)
                outsb = finp.tile([P, 2], f32)
                sgn = finp.tile([P, CHUNKS], f32)
                nc.scalar.activation(
                    out=sgn[:], in_=interp[:], func=Act.Sign, bias=0.0, scale=1.0,
                    accum_out=outsb[:, 1:2],
                )
                sq = finp.tile([P, CHUNKS], f32)
                nc.scalar.square(sq[:], interp[:])
                cube = finp.tile([P, CHUNKS], f32)
                nc.vector.tensor_tensor(out=cube[:], in0=sq[:], in1=interp[:], op=Alu.mult)
                nc.vector.tensor_reduce(out=outsb[:, 0:1], in_=cube[:], axis=X, op=Alu.add)
                nc.sync.dma_start(out=out[:], in_=outsb[:])

    nc.compile()
    return nc


def _get_nc():
    if "nc" not in _CACHE:
        _CACHE["nc"] = _build_nc()
    return _CACHE["nc"]


def _make_in_maps(pred_pos, obstacle_pos, obstacle_faces):
    pred_pos = np.ascontiguousarray(np.asarray(pred_pos, dtype=np.float32))
    obstacle_pos = np.ascontiguousarray(np.asarray(obstacle_pos, dtype=np.float32))
    faces = np.asarray(obstacle_faces).astype(np.int64)
    in_maps = []
    for c in range(NCORES):
        b, half = c // SPLIT, c % SPLIT
        pr = np.ascontiguousarray(pred_pos[b, half * NS:(half + 1) * NS])
        # gather-order indices, per piece t of 2048 faces: Q7 core q handles
        # faces [t*2048 + q*256, +256); list order j = (part, slot, corner);
        # round-robin wrapped in the core's 16 partitions:
        # gidx[16q+r, t*48 + m] = list[m*16 + r].
        fc = faces[b].reshape(NBLK, 8, 3 * FPQ * 16)   # piece, core, (q,i,k)
        gx = np.zeros((P, NBLK * 3 * FPQ), np.int16)
        for t in range(NBLK):
            for q in range(8):
                gx[16 * q:16 * q + 16, t * 3 * FPQ:(t + 1) * 3 * FPQ] = (
                    fc[t, q].reshape(3 * FPQ, 16).T.astype(np.int16)
                )
        in_maps.append({
            "pred": pr,
            "predT": np.ascontiguousarray(pr.T),
            "opos": obstacle_pos[b],
            "gidx": gx,
        })
    return in_maps


def kernel(pred_pos, obstacle_pos, obstacle_faces):
    from concourse.bass_utils import run_bass_kernel_spmd

    nc = _get_nc()
    in_maps = _make_in_maps(pred_pos, obstacle_pos, obstacle_faces)
    res = run_bass_kernel_spmd(nc, in_maps, core_ids=list(range(NCORES)))
    outs = np.stack([r["out"] for r in res.results])  # [8, 128, 2]
    loss_sum = float(outs[:, :, 0].astype(np.float64).sum())
    cnt_sum = float(outs[:, :, 1].astype(np.float64).sum())
    loss = np.float32(loss_sum / B * WEIGHT)
    perc = np.float32(cnt_sum / (B * N))
    return loss, perc
